# revision 1
# baseline (speedup 1.0000x reference)
"""Trainium2 kernel for nn_Attention_39204461478201.

The reference computes
    scores  = einsum('bqh,bkh->bqk', x, x) / sqrt(H)
    weights = softmax(scores, axis=1)          # over the q axis!
    context = einsum('bqk,bkh->bqh', weights, x)
    out     = mean(context, axis=1)
Because the softmax normalizes over axis=1 (q), every column of `weights`
sums to 1:  sum_q w[b,q,k] = 1 for all (b,k).  Therefore
    out[b,h] = (1/T) sum_q sum_k w[b,q,k] x[b,k,h]
             = (1/T) sum_k x[b,k,h] * (sum_q w[b,q,k])
             = mean(x, axis=1)[b,h]
— the attention collapses exactly to mean pooling over the time axis
(hence arch_category "pooling").

Device kernel: pure data parallel over 8 cores (2 batches/core).  Each
core streams its 8 MB slice from HBM and reduces it on the TensorEngine:
    psum[1,512] += w[128,1].T @ tile[128,512]     (PSUM-accumulated)
with w = 1/T = 2^-11.  Using float32r the PE streams 1 column/cycle
(~213 ns per [128,512] tile), so all compute hides under the DMA.

DMA layout (HW-tuned, see bench*.py):
  * rows grouped as "(p r)": partition p holds RB=16 *contiguous* rows,
    so every DMA is a fully linear HBM read (32 KB/partition chunks) —
    measured ~6% faster than the strided "(r p)" layout;
  * 1 MB DMAs alternating between the two physical HWDGE rings
    (sync + scalar sequencers) — saturates HBM at ~360 GB/s/core
    (23.3 us steady-state = the per-core HBM roofline);
  * batch 1's final DMAs shrink ([...,2,1,1] row-blocks) so the exposed
    tail after the last byte lands is just one matmul + PSUM copy + 2 KB
    output DMA.  Measured single-shot ~28.8 us/core (chained-NEFF method).
"""

import numpy as np

B, T, H = 16, 2048, 512
N_CORES = 8
B_PER = B // N_CORES     # batches per core
P = 128                  # SBUF partitions
RB = T // P              # 16 row-blocks of [128, H] per batch

# row-blocks per DMA; batch 0 hides under batch 1's stream, batch 1
# tapers so the last DMA is small (short exposed tail)
GROUPS = {0: [4, 4, 4, 4], 1: [4, 4, 4, 2, 1, 1]}

_prog_cache = {}


def _build_program(n_iters=1):
    if n_iters in _prog_cache:
        return _prog_cache[n_iters]

    import concourse.bass as bass
    import concourse.tile as tile
    from concourse import bacc, mybir

    nc = bacc.Bacc(
        "TRN2", target_bir_lowering=False, debug=False, num_devices=N_CORES
    )
    x = nc.dram_tensor("x", (B_PER, T, H), mybir.dt.float32r, kind="ExternalInput")
    out = nc.dram_tensor("out", (B_PER, H), mybir.dt.float32, kind="ExternalOutput")

    with tile.TileContext(nc) as tc:
        with (
            tc.tile_pool(name="w", bufs=1) as wpool,
            tc.tile_pool(name="xin", bufs=1) as xpool,
            tc.tile_pool(name="ps", bufs=B_PER, space=bass.MemorySpace.PSUM) as pspool,
            tc.tile_pool(name="res", bufs=B_PER) as respool,
        ):
            w = wpool.tile([P, 1], mybir.dt.float32)
            nc.vector.memset(w[:], 1.0 / T)
            w_r = w[:].bitcast(mybir.dt.float32r)
            seq = 0
            for _it in range(n_iters):
                for b in range(B_PER):
                    # partition p <- RB contiguous rows: fully linear DMA reads
                    xb = x.ap()[b].rearrange("(p r) h -> p r h", p=P)
                    ps = pspool.tile([1, H], mybir.dt.float32)
                    off = 0
                    n_done = 0
                    total = sum(GROUPS[b])
                    for i, g in enumerate(GROUPS[b]):
                        eng = nc.sync if seq % 2 == 0 else nc.scalar
                        seq += 1
                        t = xpool.tile([P, g, H], mybir.dt.float32r, tag=f"s{b}_{i}")
                        eng.dma_start(t[:], xb[:, off : off + g, :])
                        for r in range(g):
                            nc.tensor.matmul(
                                ps[:],
                                w_r,
                                t[:, r, :],
                                start=(n_done == 0),
                                stop=(n_done == total - 1),
                            )
                            n_done += 1
                        off += g
                    res = respool.tile([1, H], mybir.dt.float32)
                    nc.scalar.copy(res[:], ps[:])
                    nc.sync.dma_start(out.ap()[b : b + 1, :], res[:])
    nc.compile()
    _prog_cache[n_iters] = nc
    return nc


def kernel(lstm_out, **_unused):
    import os

    from concourse.bass_utils import run_bass_kernel_spmd

    x = np.ascontiguousarray(np.asarray(lstm_out), dtype=np.float32)
    assert x.shape == (B, T, H), x.shape
    in_maps = [{"x": x[i * B_PER : (i + 1) * B_PER]} for i in range(N_CORES)]
    nc = _build_program()
    core_ids = list(range(N_CORES))
    try:
        res = run_bass_kernel_spmd(nc, in_maps, core_ids=core_ids)
    except ModuleNotFoundError:
        # BASS_TRACE set but the axon NTFF hook isn't shipped in this
        # container (antenv.axon_hooks) — rerun with tracing disabled.
        os.environ["BASS_NEVER_TRACE"] = "1"
        res = run_bass_kernel_spmd(nc, in_maps, core_ids=core_ids)
    return np.concatenate([r["out"] for r in res.results], axis=0)



# revision 3
# speedup vs baseline: 1.0338x; 1.0338x over previous
"""Trainium2 kernel for nn_Attention_39204461478201.

The reference computes
    scores  = einsum('bqh,bkh->bqk', x, x) / sqrt(H)
    weights = softmax(scores, axis=1)          # over the q axis!
    context = einsum('bqk,bkh->bqh', weights, x)
    out     = mean(context, axis=1)
Because the softmax normalizes over axis=1 (q), every column of `weights`
sums to 1:  sum_q w[b,q,k] = 1 for all (b,k).  Therefore
    out[b,h] = (1/T) sum_q sum_k w[b,q,k] x[b,k,h]
             = (1/T) sum_k x[b,k,h] * (sum_q w[b,q,k])
             = mean(x, axis=1)[b,h]
— the attention collapses exactly to mean pooling over the time axis
(hence arch_category "pooling").

Device kernel: pure data parallel over 8 cores (2 batches/core).  Each
core streams its 8 MB slice from HBM and reduces it on the TensorEngine:
    psum[1,512] += w[128,1].T @ tile[128,512]     (PSUM-accumulated)
with w = 1/T = 2^-11.  Using float32r the PE streams 1 column/cycle
(~370 ns per [128,512] tile), comfortably under the DMA rate, so all
compute hides under the stream.

Schedule (HW-tuned via For_i-marginal benchmarking, see bench.py):
  * rows grouped as "(p r)": partition p holds RB=16 *contiguous* rows,
    so every DMA is a fully linear HBM read;
  * the two physical HWDGE rings (sync + scalar sequencers) each carry
    EXACTLY 16 of the 32 row-blocks (4 MB/ring) — a balanced split;
    steady-state stream ~23.4 us = the ~358 GB/s per-NC HBM limit
    (716 GB/s per stack shared by 2 NCs);
  * taper: 1 MB leading DMAs, then 512 KB / 256 KB, and the last TWO
    row-blocks (r14, r15 of batch 1) are column-split 256+256 across
    both rings so the final chunks are 128 KB and land simultaneously;
    the exposed tail after the last byte is one 256-col matmul, a
    PSUM->SBUF copy split across DVE+ACT (halves in parallel), and a
    2 KB output DMA (HBM write-receipt bound, ~1 us);
  * output DMAs issue on the scalar(ACT) ring right after its copy half,
    avoiding a cross-engine hop on the critical path.
Measured (For_i-marginal, paired-round median): ~0.7-1.2 us faster
single-shot than the previous [4,4,4,2,1,1] unbalanced schedule.
"""

import numpy as np

B, T, H = 16, 2048, 512
N_CORES = 8
B_PER = B // N_CORES    # batches per core
P = 128                 # SBUF partitions
RB = T // P             # 16 row-blocks of [128, H] per batch

# (batch, first row-block, n row-blocks, ring) in issue order; 16 blocks/ring
DMAS = [
    (0, 0, 4, "sync"),
    (0, 4, 4, "scalar"),
    (0, 8, 4, "sync"),
    (0, 12, 4, "scalar"),
    (1, 0, 4, "sync"),
    (1, 4, 4, "scalar"),
    (1, 8, 2, "sync"),
    (1, 10, 2, "scalar"),
    (1, 12, 1, "sync"),
    (1, 13, 1, "scalar"),
]
# batch-1 tail: row-blocks 14/15 column-split across both rings
FINAL_B = 1
FINAL = [
    (14, 0, 256, "sync"),
    (14, 256, 512, "scalar"),
    (15, 0, 256, "sync"),
    (15, 256, 512, "scalar"),
]

_prog_cache = {}


def _build_program():
    if "nc" in _prog_cache:
        return _prog_cache["nc"]

    import concourse.bass as bass
    import concourse.tile as tile
    from concourse import bacc, mybir

    nc = bacc.Bacc(
        "TRN2", target_bir_lowering=False, debug=False, num_devices=N_CORES
    )
    x = nc.dram_tensor("x", (B_PER, T, H), mybir.dt.float32r, kind="ExternalInput")
    out = nc.dram_tensor("out", (B_PER, H), mybir.dt.float32, kind="ExternalOutput")

    with tile.TileContext(nc) as tc:
        with (
            tc.tile_pool(name="w", bufs=1) as wpool,
            tc.tile_pool(name="xin", bufs=1) as xpool,
            tc.tile_pool(name="ps", bufs=B_PER, space=bass.MemorySpace.PSUM) as pspool,
            tc.tile_pool(name="res", bufs=1) as respool,
        ):
            w = wpool.tile([P, 1], mybir.dt.float32)
            nc.vector.memset(w[:], 1.0 / T)
            w_r = w[:].bitcast(mybir.dt.float32r)
            engs = {"sync": nc.sync, "scalar": nc.scalar}

            ps = {}
            started = {}
            blocks_done = {b: 0 for b in range(B_PER)}
            total_blocks = {b: RB for b in range(B_PER)}
            total_blocks[FINAL_B] -= len({r for r, _, _, _ in FINAL})

            def get_ps(b):
                if b not in ps:
                    ps[b] = pspool.tile([1, H], mybir.dt.float32, name=f"ps{b}")
                    started[b] = False
                return ps[b]

            def finish_batch(b):
                res = respool.tile(
                    [1, H], mybir.dt.float32, name=f"res{b}", tag=f"res{b}"
                )
                # split the PSUM->SBUF copy across DVE + ACT (parallel halves)
                nc.vector.tensor_copy(res[:, 0 : H // 2], ps[b][:, 0 : H // 2])
                nc.scalar.copy(res[:, H // 2 : H], ps[b][:, H // 2 : H])
                nc.scalar.dma_start(out.ap()[b : b + 1, :], res[:])

            for tag_n, (b, r0, nr, eng) in enumerate(DMAS):
                p = get_ps(b)
                xb = x.ap()[b].rearrange("(p r) h -> p r h", p=P)
                t = xpool.tile([P, nr, H], mybir.dt.float32r, tag=f"d{tag_n}")
                engs[eng].dma_start(t[:], xb[:, r0 : r0 + nr, :])
                for r in range(nr):
                    nc.tensor.matmul(
                        ps[b][:],
                        w_r,
                        t[:, r, :],
                        start=not started[b],
                        stop=(blocks_done[b] == total_blocks[b] - 1 and b != FINAL_B),
                    )
                    started[b] = True
                    blocks_done[b] += 1

            # tail: column-split chunks; stop=True on the last chunk per range
            xb = x.ap()[FINAL_B].rearrange("(p r) h -> p r h", p=P)
            p = get_ps(FINAL_B)
            tiles = []
            for r, c0, c1, eng in FINAL:
                t = xpool.tile(
                    [P, 1, c1 - c0], mybir.dt.float32r, tag=f"f{r}_{c0}"
                )
                engs[eng].dma_start(t[:], xb[:, r : r + 1, c0:c1])
                tiles.append((t, r, c0, c1))
            for idx, (t, r, c0, c1) in enumerate(tiles):
                later = any(
                    c0 < cc1 and cc0 < c1
                    for (_t, _r, cc0, cc1) in tiles[idx + 1 :]
                )
                nc.tensor.matmul(
                    p[:, c0:c1], w_r, t[:, 0, :], start=False, stop=not later
                )

            for b in range(B_PER):
                if b != FINAL_B:
                    finish_batch(b)
            finish_batch(FINAL_B)
    nc.compile()
    _prog_cache["nc"] = nc
    return nc


def kernel(lstm_out, **_unused):
    import os

    from concourse.bass_utils import run_bass_kernel_spmd

    x = np.ascontiguousarray(np.asarray(lstm_out), dtype=np.float32)
    assert x.shape == (B, T, H), x.shape
    in_maps = [{"x": x[i * B_PER : (i + 1) * B_PER]} for i in range(N_CORES)]
    nc = _build_program()
    core_ids = list(range(N_CORES))
    try:
        res = run_bass_kernel_spmd(nc, in_maps, core_ids=core_ids)
    except ModuleNotFoundError:
        # BASS_TRACE set but the axon NTFF hook isn't shipped in this
        # container (antenv.axon_hooks) — rerun with tracing disabled.
        os.environ["BASS_NEVER_TRACE"] = "1"
        res = run_bass_kernel_spmd(nc, in_maps, core_ids=core_ids)
    return np.concatenate([r["out"] for r in res.results], axis=0)
